# revision 15
# baseline (speedup 1.0000x reference)
"""Trainium2 Bass kernel for a char-CNN (embed lookup + conv1d(K=5,pad=2) + bias + maxpool).

Math: out[n, f] = max_w ( b[f] + sum_k sum_d  E[ids[n, w+k-2], d] * Wc[f, d, k] )

Strategy (pure data-parallel over 8 cores, 4096 tokens each):
  * Host-side constant folding (weights only): G[k][v, f] = sum_d E[v, d] * Wc[f, d, k].
    The embedding+conv collapses to y[n,:,w] = sum_k G[k][ids[n,w+k-2], :] + b.
  * On device, table lookup is done on the TensorEngine as one-hot matmuls with
    contraction over the vocab (96) plus a constant-ones row that carries the bias:
      - broadcast ids across partitions with a K=1 ones-matmul  -> psum [96, cols]
      - one-hot = is_equal(bcast, iota_per_partition) on the VectorE -> fp16 [97, cols]
      - 5 PSUM-accumulated matmuls against G_k (fp16), shifted along the
        char-position axis with per-tap restricted ranges (conv boundary handling)
      - reduce_max over the 16 positions on VectorE
  * Output is produced as [F, n_shard] per core; host transposes/concats shards.
"""

import numpy as np

import concourse.bass as bass
import concourse.bacc as bacc
import concourse.mybir as mybir
from concourse.tile import TileContext
from concourse.bass_utils import run_bass_kernel_spmd

# Problem shapes (hardcoded per contract)
N, W = 32768, 16
VOCAB, D, F, K = 96, 100, 100, 5
N_CORES = 8
NSH = N // N_CORES            # tokens per core = 4096
CHUNK = 128                   # tokens per inner chunk (=> 2048 one-hot cols = 4 psum banks)
NCHUNK = NSH // CHUNK         # 32
GROUP = 512                   # tokens per ids DMA
NGROUP = NSH // GROUP         # 8
CPG = GROUP // CHUNK          # chunks per group = 4
VP = VOCAB + 1                # 96 vocab rows + 1 ones row (bias)
CW = K * F + 4 + VOCAB        # packed consts width: 500 gtab + pad + iota/ones

f16 = mybir.dt.float16
f32 = mybir.dt.float32
i32 = mybir.dt.int32


def build_nc():
    nc = bacc.Bacc("TRN2", target_bir_lowering=False)

    ids_d = nc.dram_tensor("ids", [NSH, W], i32, kind="ExternalInput")
    # consts[:, 0:500] = G taps ([v, k*100+f]); [:, 500] = iota; [0, 504:600] = ones
    consts_d = nc.dram_tensor("consts", [VP, CW], f32, kind="ExternalInput")
    out_d = nc.dram_tensor("out", [F, NSH], f32, kind="ExternalOutput")

    with TileContext(nc) as tc:
        with (
            tc.tile_pool(name="consts", bufs=1) as consts,
            tc.tile_pool(name="idsp", bufs=2) as idsp,
            tc.tile_pool(name="psA", bufs=1, space="PSUM") as psA,
            tc.tile_pool(name="psB", bufs=1, space="PSUM") as psB,
        ):
            cons = consts.tile([VP, CW], f32)
            nc.sync.dma_start(out=cons, in_=consts_d[:, :])
            gtab = cons[:, 0 : K * F].rearrange("v (k f) -> v k f", k=K)
            iota_t = cons[0:VOCAB, K * F : K * F + 1]
            ones_t = cons[0:1, K * F + 4 : K * F + 4 + VOCAB]

            out_sb = consts.tile([F, NSH], f32, tag="out_sb")

            # Two persistent one-hot tiles in padded layout [VP, CHUNK, W+4]:
            # char position w lives at column w+2; pad columns {0,1,18,19} stay
            # zero forever so shifted tap reads see zero contributions at token
            # boundaries. Row 96 stays constant 1.0 (bias row, used by tap k=2
            # which only ever reads the valid region).
            o_tiles = []
            for j in range(2):
                ot = consts.tile([VP, CHUNK, W + 4], f32, tag=f"onehot{j}")
                nc.vector.memset(ot[:, :, :], 0.0)
                nc.vector.memset(ot[VOCAB : VOCAB + 1, :, :], 1.0)
                o_tiles.append(ot)

            # PE instructions may carry at most ONE sync wait. Consume the
            # consts-DMA wait with a throwaway 1x1 matmul so real matmuls
            # only ever need to wait on one new semaphore each.
            warm = psA.tile([1, 1], f32, tag="bcast")
            nc.tensor.matmul(
                warm[0:1, 0:1],
                cons[0:1, K * F + 4 : K * F + 5],
                cons[0:1, K * F + 4 : K * F + 5],
                start=True,
                stop=True,
            )

            for g in range(NGROUP):
                # ids [512, 16] i32 -> [1, 8192] f32 (SWDGE dtype-cast DMA)
                idst = idsp.tile([1, GROUP * W], f32, tag="ids")
                nc.gpsimd.dma_start(
                    out=idst[:, :],
                    in_=ids_d[g * GROUP : (g + 1) * GROUP, :],
                )
                for cc in range(CPG):
                    c = g * CPG + cc
                    o_t = o_tiles[c % 2]

                    # --- broadcast ids across 96 partitions (K=1 matmul) ---
                    bc = psA.tile([VOCAB, CHUNK, W], f32, tag="bcast")
                    for q in range(4):
                        nc.tensor.matmul(
                            bc[:, q * 32 : (q + 1) * 32, :],
                            ones_t[:, :],
                            idst[0:1, cc * CHUNK * W + q * 512 : cc * CHUNK * W + (q + 1) * 512],
                            start=True,
                            stop=True,
                        )

                    # --- one-hot: O[v, t, w+2] = (ids[t, w] == v) ---
                    nc.vector.tensor_scalar(
                        out=o_t[0:VOCAB, :, 2 : 2 + W],
                        in0=bc[:, :, :],
                        scalar1=iota_t[:, 0:1],
                        scalar2=None,
                        op0=mybir.AluOpType.is_equal,
                    )

                    # --- 5 shifted tap matmuls, accumulate in PSUM ---
                    y = psB.tile([F, CHUNK, W], f32, tag="y")
                    for k in range(K):
                        for q in range(4):
                            nc.tensor.matmul(
                                y[:, q * 32 : (q + 1) * 32, :],
                                gtab[:, k, :],
                                o_t[:, q * 32 : (q + 1) * 32, k : k + W],
                                start=(k == 0),
                                stop=(k == K - 1),
                                skip_group_check=True,
                            )

                    # --- max over the 16 char positions ---
                    nc.vector.reduce_max(
                        out=out_sb[:, c * CHUNK : (c + 1) * CHUNK],
                        in_=y[:, :, :],
                        axis=mybir.AxisListType.X,
                    )

                # stream result for this group out to DRAM
                nc.sync.dma_start(
                    out=out_d[:, g * GROUP : (g + 1) * GROUP],
                    in_=out_sb[:, g * GROUP : (g + 1) * GROUP],
                )

    nc.compile()
    return nc


def make_consts(embed_table, conv_w, conv_b):
    # G[k][v, f] = sum_d E[v, d] * Wc[f, d, k]  (computed in float64, stored fp32)
    G = np.einsum(
        "vd,fdk->kvf", embed_table.astype(np.float64), conv_w.astype(np.float64)
    )
    cons = np.zeros((VP, CW), np.float32)
    gt = np.transpose(G, (1, 0, 2)).astype(np.float32).reshape(VOCAB, K * F)
    cons[0:VOCAB, 0 : K * F] = gt
    cons[VOCAB, 2 * F : 3 * F] = conv_b.astype(np.float32)  # bias on center tap
    cons[0:VOCAB, K * F] = np.arange(VOCAB, dtype=np.float32)  # iota column
    cons[0, K * F + 4 : K * F + 4 + VOCAB] = 1.0  # ones row for bcast lhsT
    return cons


_NC_CACHE = {}

# Test-harness knobs (ignored by normal kernel() use)
TRACE = False
LAST_RESULT = None


def kernel(char_ids, embed_table, conv_w, conv_b):
    global LAST_RESULT
    char_ids = np.asarray(char_ids)
    cons = make_consts(
        np.asarray(embed_table), np.asarray(conv_w), np.asarray(conv_b)
    )

    if "nc" not in _NC_CACHE:
        _NC_CACHE["nc"] = build_nc()
    nc = _NC_CACHE["nc"]

    in_maps = []
    for c in range(N_CORES):
        shard = np.ascontiguousarray(char_ids[c * NSH : (c + 1) * NSH])
        in_maps.append({"ids": shard, "consts": cons})

    kwargs = {}
    if TRACE:
        kwargs = dict(trace=True, trace_cores=list(range(N_CORES)))
    res = run_bass_kernel_spmd(nc, in_maps, core_ids=list(range(N_CORES)), **kwargs)
    LAST_RESULT = res

    out = np.empty((N, F), np.float32)
    for c in range(N_CORES):
        out[c * NSH : (c + 1) * NSH] = res.results[c]["out"].T
    return out


# revision 24
# speedup vs baseline: 1.6103x; 1.6103x over previous
"""Trainium2 Bass kernel for a char-CNN (embed lookup + conv1d(K=5,pad=2) + bias + maxpool).

Math: out[n, f] = max_w ( b[f] + sum_k sum_d  E[ids[n, w+k-2], d] * Wc[f, d, k] )

Strategy (pure data-parallel over 8 cores, 4096 tokens each):
  * Host-side constant folding (weights only): G[k][v, f] = sum_d E[v, d] * Wc[f, d, k].
    The embedding+conv collapses to y[n,:,w] = sum_k G[k][ids[n,w+k-2], :] + b.
  * On device, table lookup is done on the TensorEngine as one-hot matmuls with
    contraction over the vocab (96) plus a constant-ones row that carries the bias:
      - broadcast ids across partitions with a K=1 ones-matmul  -> psum [96, cols]
      - one-hot = is_equal(bcast, iota_per_partition) on the VectorE -> fp16 [97, cols]
      - 5 PSUM-accumulated matmuls against G_k (fp16), shifted along the
        char-position axis with per-tap restricted ranges (conv boundary handling)
      - reduce_max over the 16 positions on VectorE
  * Output is produced as [F, n_shard] per core; host transposes/concats shards.
"""

import numpy as np

import concourse.bass as bass
import concourse.bacc as bacc
import concourse.mybir as mybir
from concourse.tile import TileContext
from concourse.bass_utils import run_bass_kernel_spmd

# Problem shapes (hardcoded per contract)
N, W = 32768, 16
VOCAB, D, F, K = 96, 100, 100, 5
N_CORES = 8
NSH = N // N_CORES            # tokens per core = 4096
UNIT = 64                     # tokens per pipeline unit (=> 1024 one-hot cols)
NUNIT = NSH // UNIT           # 64
GROUP = 512                   # tokens per ids DMA
NGROUP = NSH // GROUP         # 8
UPG = GROUP // UNIT           # units per group = 8
VP = VOCAB + 1                # 96 vocab rows + 1 ones row (bias)
CW = K * F + 4 + VOCAB        # packed consts width: 500 gtab + pad + iota/ones

f16 = mybir.dt.float16
f32 = mybir.dt.float32
f32r = mybir.dt.float32r
i32 = mybir.dt.int32


def _r(ap):
    # fp32 "raw" streaming mode: full precision, 1 col/cycle on PE (vs 4 for f32)
    return ap.bitcast(f32r)


def build_nc():
    nc = bacc.Bacc("TRN2", target_bir_lowering=False)

    ids_d = nc.dram_tensor("ids", [NSH, W], i32, kind="ExternalInput")
    # G split tables: [v, (split s, tap k), f]  s=0 -> fp16(G), s=1 -> fp16(G - hi)
    gtab_d = nc.dram_tensor("gtab", [VP, 2 * K, F], f32r, kind="ExternalInput")
    iota_d = nc.dram_tensor("iota", [VOCAB, 1], f32, kind="ExternalInput")
    ones_d = nc.dram_tensor("ones", [1, VOCAB], f32r, kind="ExternalInput")
    oinit_d = nc.dram_tensor("oinit", [VP, UNIT, W + 4], f32r, kind="ExternalInput")
    out_d = nc.dram_tensor("out", [F, NSH], f32, kind="ExternalOutput")

    with TileContext(nc) as tc:
        with (
            tc.tile_pool(name="consts", bufs=1) as consts,
            tc.tile_pool(name="idsp", bufs=2) as idsp,
            tc.tile_pool(name="psA", bufs=2, space="PSUM") as psA,
            tc.tile_pool(name="psB", bufs=2, space="PSUM") as psB,
        ):
            gtab = consts.tile([VP, 2 * K, F], f32r)
            nc.sync.dma_start(out=gtab, in_=gtab_d[:, :, :])
            iota_t = consts.tile([VOCAB, 1], f32)
            nc.sync.dma_start(out=iota_t, in_=iota_d[:, :])
            ones_t = consts.tile([1, VOCAB], f32r)
            nc.sync.dma_start(out=ones_t, in_=ones_d[:, :])

            out_sb = consts.tile([F, NSH], f32, tag="out_sb")

            # Two persistent one-hot tiles, padded layout [VP, UNIT, W+4]:
            # char position w at column w+2, pad columns {0,1,18,19} stay zero,
            # row 96 constant 1.0 (bias row, consumed only by center tap).
            o_tiles = []
            for j in range(2):
                ot = consts.tile([VP, UNIT, W + 4], f32r, tag=f"onehot{j}")
                nc.sync.dma_start(out=ot[:, :, :], in_=oinit_d[:, :, :])
                o_tiles.append(ot)

            for g in range(NGROUP):
                # ids [512, 16] i32 -> [1, 8192] fp16 (SWDGE dtype-cast DMA)
                idst = idsp.tile([1, GROUP * W], f32r, tag="ids")
                nc.gpsimd.dma_start(
                    out=idst[:, :],
                    in_=ids_d[g * GROUP : (g + 1) * GROUP, :],
                )
                for uu in range(UPG):
                    u = g * UPG + uu
                    o_t = o_tiles[u % 2]

                    # broadcast ids across 96 partitions (K=1 matmul, N=1024)
                    bc = psA.tile([VOCAB, UNIT, W], f32, tag="bcast")
                    for h in range(2):
                        nc.tensor.matmul(
                            bc[:, h * 32 : (h + 1) * 32, :],
                            ones_t[:, :],
                            idst[
                                0:1,
                                (uu * UNIT + h * 32) * W : (uu * UNIT + (h + 1) * 32) * W,
                            ],
                            start=True,
                            stop=True,
                        )

                    # one-hot: O[v, t, w+2] = (ids[t, w] == v)
                    nc.vector.tensor_scalar(
                        out=o_t[0:VOCAB, :, 2 : 2 + W],
                        in0=bc[:, :, :],
                        scalar1=iota_t[:, 0:1],
                        scalar2=None,
                        op0=mybir.AluOpType.is_equal,
                    )

                    # 5 taps x 2 precision splits, PSUM-accumulated (N=1024 each)
                    y = psB.tile([F, UNIT, W], f32, tag="y")
                    first = True
                    for s in range(2):
                        for k in range(K):
                            for h in range(2):
                                nc.tensor.matmul(
                                    y[:, h * 32 : (h + 1) * 32, :],
                                    gtab[:, s * K + k, :],
                                    o_t[:, h * 32 : (h + 1) * 32, k : k + W],
                                    start=first,
                                    stop=(s == 1 and k == K - 1),
                                    skip_group_check=True,
                                )
                            first = False

                    # max over the 16 char positions
                    nc.vector.reduce_max(
                        out=out_sb[:, u * UNIT : (u + 1) * UNIT],
                        in_=y[:, :, :],
                        axis=mybir.AxisListType.X,
                    )

                # stream this group's result out to DRAM
                nc.sync.dma_start(
                    out=out_d[:, g * GROUP : (g + 1) * GROUP],
                    in_=out_sb[:, g * GROUP : (g + 1) * GROUP],
                )

    nc.compile()
    return nc


def _round_f32r(x):
    """FP32R keeps 11 explicit mantissa bits (low 12 bits of fp32 zeroed)."""
    b = np.asarray(x, np.float32).view(np.uint32)
    b = (b + 0x800) & np.uint32(0xFFFFF000)
    return b.view(np.float32)


def make_consts(embed_table, conv_w, conv_b):
    # G[k][v, f] = sum_d E[v, d] * Wc[f, d, k] in float64, split hi/lo f32r
    G = np.einsum(
        "vd,fdk->kvf", embed_table.astype(np.float64), conv_w.astype(np.float64)
    )
    Gf = np.zeros((K, VP, F), np.float64)
    Gf[:, 0:VOCAB, :] = G
    Gf[2, VOCAB, :] = conv_b.astype(np.float64)  # bias rides center tap
    hi = _round_f32r(Gf.astype(np.float32))
    lo = _round_f32r((Gf - hi.astype(np.float64)).astype(np.float32))
    gtab = np.zeros((VP, 2 * K, F), np.float32)
    gtab[:, 0:K, :] = np.transpose(hi, (1, 0, 2))
    gtab[:, K : 2 * K, :] = np.transpose(lo, (1, 0, 2))
    iota = np.arange(VOCAB, dtype=np.float32).reshape(VOCAB, 1)
    ones = np.ones((1, VOCAB), np.float32)
    oinit = np.zeros((VP, UNIT, W + 4), np.float32)
    oinit[VOCAB, :, :] = 1.0
    return gtab, iota, ones, oinit


_NC_CACHE = {}

# Test-harness knobs (ignored by normal kernel() use)
TRACE = False
LAST_RESULT = None


def kernel(char_ids, embed_table, conv_w, conv_b):
    global LAST_RESULT
    char_ids = np.asarray(char_ids)
    gtab, iota, ones, oinit = make_consts(
        np.asarray(embed_table), np.asarray(conv_w), np.asarray(conv_b)
    )

    if "nc" not in _NC_CACHE:
        _NC_CACHE["nc"] = build_nc()
    nc = _NC_CACHE["nc"]

    in_maps = []
    for c in range(N_CORES):
        shard = np.ascontiguousarray(char_ids[c * NSH : (c + 1) * NSH])
        in_maps.append(
            {"ids": shard, "gtab": gtab, "iota": iota, "ones": ones, "oinit": oinit}
        )

    kwargs = {}
    if TRACE:
        kwargs = dict(trace=True, trace_cores=list(range(N_CORES)))
    res = run_bass_kernel_spmd(nc, in_maps, core_ids=list(range(N_CORES)), **kwargs)
    LAST_RESULT = res

    out = np.empty((N, F), np.float32)
    for c in range(N_CORES):
        out[c * NSH : (c + 1) * NSH] = res.results[c]["out"].T
    return out


# revision 27
# speedup vs baseline: 1.9651x; 1.2203x over previous
"""Trainium2 Bass kernel for a char-CNN (embed lookup + conv1d(K=5,pad=2) + bias + maxpool).

Math: out[n, f] = max_w ( b[f] + sum_k sum_d  E[ids[n, w+k-2], d] * Wc[f, d, k] )

Strategy (pure data-parallel over 8 cores, 4096 tokens each):
  * Host-side constant folding (weights only): G[k][v, f] = sum_d E[v, d] * Wc[f, d, k].
    The embedding+conv collapses to y[n,:,w] = sum_k G[k][ids[n,w+k-2], :] + b.
  * On device, table lookup is done on the TensorEngine as one-hot matmuls with
    contraction over the vocab (96) plus a constant-ones row that carries the bias:
      - broadcast ids across partitions with a K=1 ones-matmul  -> psum [96, cols]
      - one-hot = is_equal(bcast, iota_per_partition) on the VectorE -> fp16 [97, cols]
      - 5 PSUM-accumulated matmuls against G_k (fp16), shifted along the
        char-position axis with per-tap restricted ranges (conv boundary handling)
      - reduce_max over the 16 positions on VectorE
  * Output is produced as [F, n_shard] per core; host transposes/concats shards.
"""

import numpy as np

import concourse.bass as bass
import concourse.bacc as bacc
import concourse.mybir as mybir
from concourse.tile import TileContext
from concourse.bass_utils import run_bass_kernel_spmd

# Problem shapes (hardcoded per contract)
N, W = 32768, 16
VOCAB, D, F, K = 96, 100, 100, 5
N_CORES = 8
NSH = N // N_CORES            # tokens per core = 4096
UNIT = 64                     # tokens per pipeline unit (=> 1024 one-hot cols)
NUNIT = NSH // UNIT           # 64
GROUP = 512                   # tokens per ids DMA
NGROUP = NSH // GROUP         # 8
UPG = GROUP // UNIT           # units per group = 8
VP = VOCAB + 1                # 96 vocab rows + 1 ones row (bias)
CW = K * F + 4 + VOCAB        # packed consts width: 500 gtab + pad + iota/ones

f16 = mybir.dt.float16
f32 = mybir.dt.float32
f32r = mybir.dt.float32r
i32 = mybir.dt.int32


def _r(ap):
    # fp32 "raw" streaming mode: full precision, 1 col/cycle on PE (vs 4 for f32)
    return ap.bitcast(f32r)


def build_nc():
    nc = bacc.Bacc("TRN2", target_bir_lowering=False)

    ids_d = nc.dram_tensor("ids", [NSH, W], i32, kind="ExternalInput")
    # G split tables: [v, (split s, tap k), f]  s=0 -> fp16(G), s=1 -> fp16(G - hi)
    gtab_d = nc.dram_tensor("gtab", [VP, 2 * K, F], f32r, kind="ExternalInput")
    iota_d = nc.dram_tensor("iota", [VOCAB, 1], f32, kind="ExternalInput")
    ones_d = nc.dram_tensor("ones", [1, VOCAB], f32r, kind="ExternalInput")
    oinit_d = nc.dram_tensor("oinit", [VP, UNIT, W + 4], f32r, kind="ExternalInput")
    out_d = nc.dram_tensor("out", [F, NSH], f32, kind="ExternalOutput")

    with TileContext(nc) as tc:
        with (
            tc.tile_pool(name="consts", bufs=1) as consts,
            tc.tile_pool(name="idsp", bufs=3) as idsp,
            tc.tile_pool(name="psA", bufs=2, space="PSUM") as psA,
            tc.tile_pool(name="psB", bufs=2, space="PSUM") as psB,
        ):
            gtab = consts.tile([VP, 2 * K, F], f32r)
            nc.sync.dma_start(out=gtab, in_=gtab_d[:, :, :])
            iota_t = consts.tile([VOCAB, 1], f32)
            nc.sync.dma_start(out=iota_t, in_=iota_d[:, :])
            ones_t = consts.tile([1, VOCAB], f32r)
            nc.sync.dma_start(out=ones_t, in_=ones_d[:, :])

            out_sb = consts.tile([F, NSH], f32, tag="out_sb")

            # Two persistent one-hot tiles, padded layout [VP, UNIT, W+4]:
            # char position w at column w+2, pad columns {0,1,18,19} stay zero,
            # row 96 constant 1.0 (bias row, consumed only by center tap).
            o_tiles = []
            for j in range(2):
                ot = consts.tile([VP, UNIT, W + 4], f32r, tag=f"onehot{j}")
                nc.sync.dma_start(out=ot[:, :, :], in_=oinit_d[:, :, :])
                o_tiles.append(ot)

            ids_tiles = {}

            def load_ids(g):
                idst = idsp.tile([1, GROUP * W], f32r, tag="ids")
                nc.gpsimd.dma_start(
                    out=idst[:, :],
                    in_=ids_d[g * GROUP : (g + 1) * GROUP, :],
                )
                ids_tiles[g] = idst

            def bcast(u):
                # broadcast ids across 96 partitions (K=1 matmul) + one-hot
                g, uu = divmod(u, UPG)
                idst = ids_tiles[g]
                bc = psA.tile([VOCAB, UNIT, W], f32, tag="bcast")
                for h in range(2):
                    c0 = (uu * UNIT + h * 32) * W
                    nc.tensor.matmul(
                        bc[:, h * 32 : (h + 1) * 32, :],
                        ones_t[:, :],
                        idst[0:1, c0 : c0 + 512],
                        start=True,
                        stop=True,
                    )
                # one-hot: O[v, t, w+2] = (ids[t, w] == v)
                o_t = o_tiles[u % 2]
                nc.vector.tensor_scalar(
                    out=o_t[0:VOCAB, :, 2 : 2 + W],
                    in0=bc[:, :, :],
                    scalar1=iota_t[:, 0:1],
                    scalar2=None,
                    op0=mybir.AluOpType.is_equal,
                )

            load_ids(0)
            bcast(0)
            for u in range(NUNIT):
                g, uu = divmod(u, UPG)
                if uu == 0 and g + 1 < NGROUP:
                    load_ids(g + 1)
                # emit next unit's bcast+one-hot BEFORE this unit's taps so the
                # in-order PE queue never stalls waiting on the DVE is_equal.
                if u + 1 < NUNIT:
                    bcast(u + 1)

                o_t = o_tiles[u % 2]
                # 5 taps x 2 precision splits, PSUM-accumulated (N=512 each)
                y = psB.tile([F, UNIT, W], f32, tag="y")
                first = True
                for s in range(2):
                    for k in range(K):
                        for h in range(2):
                            nc.tensor.matmul(
                                y[:, h * 32 : (h + 1) * 32, :],
                                gtab[:, s * K + k, :],
                                o_t[:, h * 32 : (h + 1) * 32, k : k + W],
                                start=first,
                                stop=(s == 1 and k == K - 1),
                                skip_group_check=True,
                            )
                        first = False

                # max over the 16 char positions
                nc.vector.reduce_max(
                    out=out_sb[:, u * UNIT : (u + 1) * UNIT],
                    in_=y[:, :, :],
                    axis=mybir.AxisListType.X,
                )

                if uu == UPG - 1:
                    # stream this group's result out to DRAM
                    nc.sync.dma_start(
                        out=out_d[:, g * GROUP : (g + 1) * GROUP],
                        in_=out_sb[:, g * GROUP : (g + 1) * GROUP],
                    )

    nc.compile()
    return nc


def _round_f32r(x):
    """FP32R keeps 11 explicit mantissa bits (low 12 bits of fp32 zeroed)."""
    b = np.asarray(x, np.float32).view(np.uint32)
    b = (b + 0x800) & np.uint32(0xFFFFF000)
    return b.view(np.float32)


def make_consts(embed_table, conv_w, conv_b):
    # G[k][v, f] = sum_d E[v, d] * Wc[f, d, k] in float64, split hi/lo f32r
    G = np.einsum(
        "vd,fdk->kvf", embed_table.astype(np.float64), conv_w.astype(np.float64)
    )
    Gf = np.zeros((K, VP, F), np.float64)
    Gf[:, 0:VOCAB, :] = G
    Gf[2, VOCAB, :] = conv_b.astype(np.float64)  # bias rides center tap
    hi = _round_f32r(Gf.astype(np.float32))
    lo = _round_f32r((Gf - hi.astype(np.float64)).astype(np.float32))
    gtab = np.zeros((VP, 2 * K, F), np.float32)
    gtab[:, 0:K, :] = np.transpose(hi, (1, 0, 2))
    gtab[:, K : 2 * K, :] = np.transpose(lo, (1, 0, 2))
    iota = np.arange(VOCAB, dtype=np.float32).reshape(VOCAB, 1)
    ones = np.ones((1, VOCAB), np.float32)
    oinit = np.zeros((VP, UNIT, W + 4), np.float32)
    oinit[VOCAB, :, :] = 1.0
    return gtab, iota, ones, oinit


_NC_CACHE = {}

# Test-harness knobs (ignored by normal kernel() use)
TRACE = False
LAST_RESULT = None


def kernel(char_ids, embed_table, conv_w, conv_b):
    global LAST_RESULT
    char_ids = np.asarray(char_ids)
    gtab, iota, ones, oinit = make_consts(
        np.asarray(embed_table), np.asarray(conv_w), np.asarray(conv_b)
    )

    if "nc" not in _NC_CACHE:
        _NC_CACHE["nc"] = build_nc()
    nc = _NC_CACHE["nc"]

    in_maps = []
    for c in range(N_CORES):
        shard = np.ascontiguousarray(char_ids[c * NSH : (c + 1) * NSH])
        in_maps.append(
            {"ids": shard, "gtab": gtab, "iota": iota, "ones": ones, "oinit": oinit}
        )

    kwargs = {}
    if TRACE:
        kwargs = dict(trace=True, trace_cores=list(range(N_CORES)))
    res = run_bass_kernel_spmd(nc, in_maps, core_ids=list(range(N_CORES)), **kwargs)
    LAST_RESULT = res

    out = np.empty((N, F), np.float32)
    for c in range(N_CORES):
        out[c * NSH : (c + 1) * NSH] = res.results[c]["out"].T
    return out


# revision 29
# speedup vs baseline: 2.1304x; 1.0841x over previous
"""Trainium2 Bass kernel for a char-CNN (embed lookup + conv1d(K=5,pad=2) + bias + maxpool).

Math: out[n, f] = max_w ( b[f] + sum_k sum_d  E[ids[n, w+k-2], d] * Wc[f, d, k] )

Strategy (pure data-parallel over 8 cores, 4096 tokens each):
  * Host-side constant folding (weights only): G[k][v, f] = sum_d E[v, d] * Wc[f, d, k].
    The embedding+conv collapses to y[n,:,w] = sum_k G[k][ids[n,w+k-2], :] + b.
  * On device, table lookup is done on the TensorEngine as one-hot matmuls with
    contraction over the vocab (96) plus a constant-ones row that carries the bias:
      - broadcast ids across partitions with a K=1 ones-matmul  -> psum [96, cols]
      - one-hot = is_equal(bcast, iota_per_partition) on the VectorE -> fp16 [97, cols]
      - 5 PSUM-accumulated matmuls against G_k (fp16), shifted along the
        char-position axis with per-tap restricted ranges (conv boundary handling)
      - reduce_max over the 16 positions on VectorE
  * Output is produced as [F, n_shard] per core; host transposes/concats shards.
"""

import numpy as np

import concourse.bass as bass
import concourse.bacc as bacc
import concourse.mybir as mybir
from concourse.tile import TileContext
from concourse.bass_utils import run_bass_kernel_spmd

# Problem shapes (hardcoded per contract)
N, W = 32768, 16
VOCAB, D, F, K = 96, 100, 100, 5
N_CORES = 8
NSH = N // N_CORES            # tokens per core = 4096
UNIT = 64                     # tokens per pipeline unit (=> 1024 one-hot cols)
NUNIT = NSH // UNIT           # 64
GROUP = 512                   # tokens per ids DMA
NGROUP = NSH // GROUP         # 8
UPG = GROUP // UNIT           # units per group = 8
VP = VOCAB + 1                # 96 vocab rows + 1 ones row (bias)
CW = K * F + 4 + VOCAB        # packed consts width: 500 gtab + pad + iota/ones

f16 = mybir.dt.float16
f32 = mybir.dt.float32
f32r = mybir.dt.float32r
i32 = mybir.dt.int32


def _r(ap):
    # fp32 "raw" streaming mode: full precision, 1 col/cycle on PE (vs 4 for f32)
    return ap.bitcast(f32r)


def build_nc():
    nc = bacc.Bacc("TRN2", target_bir_lowering=False)

    ids_d = nc.dram_tensor("ids", [NSH, W], i32, kind="ExternalInput")
    # G split tables: [v, (split s, tap k), f]  s=0 -> fp16(G), s=1 -> fp16(G - hi)
    gtab_d = nc.dram_tensor("gtab", [VP, 2 * K, F], f32r, kind="ExternalInput")
    iota_d = nc.dram_tensor("iota", [VOCAB, 1], f32, kind="ExternalInput")
    ones_d = nc.dram_tensor("ones", [1, VOCAB], f32r, kind="ExternalInput")
    oinit_d = nc.dram_tensor("oinit", [VP, W + 4, UNIT], f32r, kind="ExternalInput")
    out_d = nc.dram_tensor("out", [F, NSH], f32, kind="ExternalOutput")

    with TileContext(nc) as tc:
        with (
            tc.tile_pool(name="consts", bufs=1) as consts,
            tc.tile_pool(name="idsp", bufs=3) as idsp,
            tc.tile_pool(name="psA", bufs=2, space="PSUM") as psA,
            tc.tile_pool(name="psB", bufs=2, space="PSUM") as psB,
        ):
            gtab = consts.tile([VP, 2 * K, F], f32r)
            nc.sync.dma_start(out=gtab, in_=gtab_d[:, :, :])
            iota_t = consts.tile([VOCAB, 1], f32)
            nc.sync.dma_start(out=iota_t, in_=iota_d[:, :])
            ones_t = consts.tile([1, VOCAB], f32r)
            nc.sync.dma_start(out=ones_t, in_=ones_d[:, :])

            out_sb = consts.tile([F, NSH], f32, tag="out_sb")

            # Two persistent one-hot tiles, padded layout [VP, UNIT, W+4]:
            # char position w at column w+2, pad columns {0,1,18,19} stay zero,
            # row 96 constant 1.0 (bias row, consumed only by center tap).
            o_tiles = []
            for j in range(2):
                ot = consts.tile([VP, W + 4, UNIT], f32r, tag=f"onehot{j}")
                nc.sync.dma_start(out=ot[:, :, :], in_=oinit_d[:, :, :])
                o_tiles.append(ot)

            ids_tiles = {}

            def load_ids(g):
                idst = idsp.tile([1, GROUP * W], f32r, tag="ids")
                nc.gpsimd.dma_start(
                    out=idst[:, :],
                    in_=ids_d[g * GROUP : (g + 1) * GROUP, :],
                )
                ids_tiles[g] = idst

            def bcast(u):
                # broadcast ids across 96 partitions (K=1 matmul) + one-hot
                g, uu = divmod(u, UPG)
                idst = ids_tiles[g]
                bc = psA.tile([VOCAB, UNIT, W], f32, tag="bcast")
                for h in range(2):
                    c0 = (uu * UNIT + h * 32) * W
                    nc.tensor.matmul(
                        bc[:, h * 32 : (h + 1) * 32, :],
                        ones_t[:, :],
                        idst[0:1, c0 : c0 + 512],
                        start=True,
                        stop=True,
                    )
                # one-hot: O[v, t, w+2] = (ids[t, w] == v)
                o_t = o_tiles[u % 2]
                nc.vector.tensor_scalar(
                    out=o_t[0:VOCAB, 2 : 2 + W, :].rearrange("v p t -> v t p"),
                    in0=bc[:, :, :],
                    scalar1=iota_t[:, 0:1],
                    scalar2=None,
                    op0=mybir.AluOpType.is_equal,
                )

            load_ids(0)
            bcast(0)
            for u in range(NUNIT):
                g, uu = divmod(u, UPG)
                if uu == 0 and g + 1 < NGROUP:
                    load_ids(g + 1)
                # emit next unit's bcast+one-hot BEFORE this unit's taps so the
                # in-order PE queue never stalls waiting on the DVE is_equal.
                if u + 1 < NUNIT:
                    bcast(u + 1)

                o_t = o_tiles[u % 2]
                # 5 taps x 2 precision splits, PSUM-accumulated (N=512 each)
                ys = [psB.tile([F, W, 32], f32, tag=f"y{h}", name=f"y{h}") for h in range(2)]
                first = True
                for s in range(2):
                    for k in range(K):
                        for h in range(2):
                            nc.tensor.matmul(
                                ys[h][:, :, :],
                                gtab[:, s * K + k, :],
                                o_t[:, k : k + W, h * 32 : (h + 1) * 32],
                                start=first,
                                stop=(s == 1 and k == K - 1),
                                skip_group_check=True,
                            )
                        first = False

                # max over the 16 char positions
                for h in range(2):
                    nc.vector.reduce_max(
                        out=out_sb[:, u * UNIT + h * 32 : u * UNIT + (h + 1) * 32],
                        in_=ys[h].rearrange("f w t -> f t w"),
                        axis=mybir.AxisListType.X,
                    )

                if uu == UPG - 1:
                    # stream this group's result out to DRAM
                    nc.sync.dma_start(
                        out=out_d[:, g * GROUP : (g + 1) * GROUP],
                        in_=out_sb[:, g * GROUP : (g + 1) * GROUP],
                    )

    nc.compile()
    return nc


def _round_f32r(x):
    """FP32R keeps 11 explicit mantissa bits (low 12 bits of fp32 zeroed)."""
    b = np.asarray(x, np.float32).view(np.uint32)
    b = (b + 0x800) & np.uint32(0xFFFFF000)
    return b.view(np.float32)


def make_consts(embed_table, conv_w, conv_b):
    # G[k][v, f] = sum_d E[v, d] * Wc[f, d, k] in float64, split hi/lo f32r
    G = np.einsum(
        "vd,fdk->kvf", embed_table.astype(np.float64), conv_w.astype(np.float64)
    )
    Gf = np.zeros((K, VP, F), np.float64)
    Gf[:, 0:VOCAB, :] = G
    Gf[2, VOCAB, :] = conv_b.astype(np.float64)  # bias rides center tap
    hi = _round_f32r(Gf.astype(np.float32))
    lo = _round_f32r((Gf - hi.astype(np.float64)).astype(np.float32))
    gtab = np.zeros((VP, 2 * K, F), np.float32)
    gtab[:, 0:K, :] = np.transpose(hi, (1, 0, 2))
    gtab[:, K : 2 * K, :] = np.transpose(lo, (1, 0, 2))
    iota = np.arange(VOCAB, dtype=np.float32).reshape(VOCAB, 1)
    ones = np.ones((1, VOCAB), np.float32)
    oinit = np.zeros((VP, W + 4, UNIT), np.float32)
    oinit[VOCAB, :, :] = 1.0
    return gtab, iota, ones, oinit


_NC_CACHE = {}

# Test-harness knobs (ignored by normal kernel() use)
TRACE = False
LAST_RESULT = None


def kernel(char_ids, embed_table, conv_w, conv_b):
    global LAST_RESULT
    char_ids = np.asarray(char_ids)
    gtab, iota, ones, oinit = make_consts(
        np.asarray(embed_table), np.asarray(conv_w), np.asarray(conv_b)
    )

    if "nc" not in _NC_CACHE:
        _NC_CACHE["nc"] = build_nc()
    nc = _NC_CACHE["nc"]

    in_maps = []
    for c in range(N_CORES):
        shard = np.ascontiguousarray(char_ids[c * NSH : (c + 1) * NSH])
        in_maps.append(
            {"ids": shard, "gtab": gtab, "iota": iota, "ones": ones, "oinit": oinit}
        )

    kwargs = {}
    if TRACE:
        kwargs = dict(trace=True, trace_cores=list(range(N_CORES)))
    res = run_bass_kernel_spmd(nc, in_maps, core_ids=list(range(N_CORES)), **kwargs)
    LAST_RESULT = res

    out = np.empty((N, F), np.float32)
    for c in range(N_CORES):
        out[c * NSH : (c + 1) * NSH] = res.results[c]["out"].T
    return out
